# revision 32
# baseline (speedup 1.0000x reference)
"""Distributed multi-head causal attention for 8 TRN2 NeuronCores.

Problem: residual [2, 2048, 2048] f32 -> (residual, attn_out [2, 2048, 2048])
  q/k/v = residual @ W_{Q,K,V} + b  (16 heads, d_head 128)
  scores = q k^T / sqrt(128), causal mask, softmax
  out = (pattern @ v) @ W_O + b_O

Sharding: tensor-parallel over heads. Core c computes QKV projections and
attention for heads 2c, 2c+1 over both batches, producing z^T (the
pre-output-projection activations). Two 8-core AllToAlls (one per local
head) redistribute z^T from head-sharded to position-sharded: shard j covers
positions [512*j, 512*(j+1)) of the flattened [batch*seq] axis. After the
A2A each core holds all 16 heads for its own 512 positions and computes the
output projection for just those rows. The host concatenates the 8 shards.

All matmuls in bf16 (inputs pre-cast and pre-packed on host so every load is
one large contiguous DMA), accumulation f32 in PSUM:
  Q^T/K^T [dh, pos] = W^T X^T    (lhsT = W [model, dh], rhs = X^T)
  V [pos, dh*hpc]   = X W_V      (lhsT = X^T tile,      rhs = W_V heads)
  S^T [k, q]        = K Q^T      (lhsT = K^T tile,      rhs = Q^T)
  z^T [dh, q]       = V^T P^T    (lhsT = V tile,        rhs = P^T = exp(S^T))
  den [*, q]        = J acc      (lhsT = all-ones,      rhs = sum_k P^T)
  out [pos, m]      = z W_O      (lhsT = z^T tile,      rhs = W_O)

The emission driver software-pipelines the phases on the single PE queue:
attention chunks (h, b, qc) become runnable as soon as phase 1 has produced
q/k/v for (b, <=qc) (causality: chunk qc only attends keys < (qc+1)*512), so
attention matmuls interleave with later projection units and the attention
phase's scalar-engine exps and vector-engine denominator accumulations hide
under projection PE work. The AV pass is one full-width [128, 512] matmul
per k-tile; softmax denominators come from vector adds of the exp panels
(acc = sum_k P^T) reduced+broadcast by a single all-ones matmul per chunk.
Pass E of the output projection (first-A2A heads) starts the moment the last
attention matmul retires; pass O adds the second-A2A heads.
"""

import numpy as np
import ml_dtypes

import concourse.bass as bass
import concourse.tile as tile
from concourse import bacc, mybir
from concourse.bass_utils import run_bass_kernel_spmd
from concourse.tile_rust import add_dep_helper

BF16 = mybir.dt.bfloat16
F32 = mybir.dt.float32
NP_BF16 = ml_dtypes.bfloat16

FULL = dict(n_heads=16, d_model=2048, d_head=128, batch=2, seq=2048, n_cores=8)
ATTN_SCALE = float(np.sqrt(128.0))


def _derived(cfg):
    d = dict(cfg)
    d["hpc"] = d["n_heads"] // d["n_cores"]             # heads per core
    d["rows"] = d["batch"] * d["seq"] // d["n_cores"]   # out rows per core
    d["qc_size"] = d["rows"]                            # q-chunk == A2A shard
    assert d["qc_size"] <= 512
    d["n_qc"] = d["seq"] // d["qc_size"]                # q chunks per batch
    d["n_kb"] = d["seq"] // 128                         # k blocks per batch
    d["n_mb"] = d["d_model"] // 128                     # model-dim blocks
    d["n_dg"] = d["qc_size"] // 128                     # diag offsets per chunk
    d["n_mc"] = d["d_model"] // 512                     # out m-chunks
    d["n_pb"] = d["rows"] // 128                        # out pos-blocks
    assert d["n_qc"] * d["batch"] == d["n_cores"]
    assert d["d_head"] == 128
    return d


def build_graph(cfg=FULL, enable_asserts=False):
    c = _derived(cfg)
    hpc, QC = c["hpc"], c["qc_size"]
    n_qc, n_kb, n_mb, n_dg = c["n_qc"], c["n_kb"], c["n_mb"], c["n_dg"]
    n_mc, n_pb, rows = c["n_mc"], c["n_pb"], c["rows"]
    n_heads, d_model, seq = c["n_heads"], c["d_model"], c["seq"]
    batch, n_cores = c["batch"], c["n_cores"]
    dpb = QC // 128
    MC = 512

    nc = bacc.Bacc("TRN2", target_bir_lowering=False, debug=False,
                   enable_asserts=enable_asserts, num_devices=n_cores)

    # all inputs pre-packed on host into [128, ...] partition-major layouts
    xt_d = nc.dram_tensor("xt", [128, batch, n_qc, n_mb, QC], BF16,
                          kind="ExternalInput")
    wq_d = nc.dram_tensor("wq", [128, hpc, n_mb, 128], BF16, kind="ExternalInput")
    wk_d = nc.dram_tensor("wk", [128, hpc, n_mb, 128], BF16, kind="ExternalInput")
    wv_d = nc.dram_tensor("wv", [128, n_mb, hpc * 128], BF16, kind="ExternalInput")
    wo_d = nc.dram_tensor("wo", [128, n_heads, d_model], BF16, kind="ExternalInput")
    bq_d = nc.dram_tensor("bq", [128, hpc], F32, kind="ExternalInput")
    bk_d = nc.dram_tensor("bk", [128, hpc], F32, kind="ExternalInput")
    bv_d = nc.dram_tensor("bv", [hpc * 128], F32, kind="ExternalInput")
    mk_d = nc.dram_tensor("mk", [128, n_dg, QC], BF16, kind="ExternalInput")
    # bf16 output halves the final DMA tail; the host adds b_O in f32
    out_d = nc.dram_tensor("out", [rows, d_model], BF16, kind="ExternalOutput")

    rg = [list(range(n_cores))]
    Exp = mybir.ActivationFunctionType.Exp

    with tile.TileContext(nc) as tc:
        with (
            tc.tile_pool(name="stat", bufs=1) as stat,
            tc.tile_pool(name="xin", bufs=2) as xin,
            tc.tile_pool(name="work", bufs=3) as work,
            tc.tile_pool(name="ps", bufs=2, space="PSUM") as ps,
            tc.tile_pool(name="dram", bufs=1, space="DRAM") as dram,
        ):
            wq_sb = stat.tile([128, hpc, n_mb, 128], BF16)
            wk_sb = stat.tile([128, hpc, n_mb, 128], BF16)
            wv_sb = stat.tile([128, n_mb, hpc * 128], BF16, tag="wvzf")
            qt_sb = stat.tile([128, batch, hpc, seq], BF16)
            kt_sb = stat.tile([128, batch, hpc, seq], BF16)
            v_sb = stat.tile([128, batch, n_kb, hpc, 128], BF16)
            bq_sb = stat.tile([128, hpc], F32)
            bk_sb = stat.tile([128, hpc], F32)
            vb_sb = stat.tile([128, hpc * 128], F32)
            mk_sb = stat.tile([128, n_dg, QC], BF16)
            ones_sb = stat.tile([128, 128], BF16)
            # h-major so each A2A's zf tiles land with ONE strided DMA
            zf_sb = stat.tile([128, hpc, n_cores, rows], BF16, tag="wvzf")

            a2a_in = [dram.tile([n_cores, 128, rows], BF16, name=f"a2ai{h}")
                      for h in range(hpc)]
            a2a_out = [dram.tile([n_cores, 128, rows], BF16, name=f"a2ao{h}")
                       for h in range(hpc)]

            # big weights all on the scalar HWDGE queue as single descriptors
            # (issue cost ~0.6us each), ordered to match first consumption
            # (Q h0, K h0, V, Q h1, K h1); small consts on gpsimd; xt chunks
            # stream on sync alone so chunk N+1 is never stuck behind a
            # weight load
            nc.scalar.dma_start(out=wq_sb[:, 0], in_=wq_d[:, 0])
            nc.scalar.dma_start(out=wv_sb, in_=wv_d[:])
            for h in range(1, hpc):
                nc.scalar.dma_start(out=wq_sb[:, h], in_=wq_d[:, h])
            nc.gpsimd.dma_start(out=bq_sb, in_=bq_d[:])
            nc.gpsimd.dma_start(out=bk_sb, in_=bk_d[:])
            # K weights ride the otherwise-idle gpsimd ring so the first K
            # chains never queue behind the Q/V loads on scalar
            nc.gpsimd.dma_start(out=wk_sb, in_=wk_d[:])
            bv_ap = bv_d.ap()
            nc.gpsimd.dma_start(
                out=vb_sb,
                in_=bass.AP(tensor=bv_ap.tensor, offset=bv_ap.offset,
                            ap=[[0, 128]] + list(bv_ap.ap)))
            nc.gpsimd.dma_start(out=mk_sb, in_=mk_d[:])
            nc.vector.memset(ones_sb, 1.0)

            # PE warmup: ~40 tiny matmuls bridge the gap between the engine
            # barrier and the first weight arrival so the p-state ramp (3us
            # of continuous busy to reach max clock) runs on dummy work
            for w in range(40):
                dmy = ps.tile([128, 128], F32, tag="A", name=f"wrm{w}")
                nc.tensor.matmul(dmy, ones_sb, ones_sb,
                                 start=True, stop=True)

            wo_tiles = {}
            last_exp = [None]
            markers = set()
            ship_n = [0] * hpc

            def emit_cc(h):
                nc.gpsimd.collective_compute(
                    "AllToAll", mybir.AluOpType.bypass, replica_groups=rg,
                    ins=[a2a_in[h].opt()], outs=[a2a_out[h].opt()])

            def load_zf(h, eng):
                # one strided descriptor: dest [128p, cores, rows] from the
                # [cores, 128, rows] A2A result
                src = a2a_out[h][:]
                ap3 = bass.AP(tensor=src.tensor, offset=src.offset,
                              ap=[[rows, 128], [128 * rows, n_cores],
                                  [1, rows]])
                d = eng.dma_start(out=zf_sb[:, h], in_=ap3)
                if eng is nc.scalar and last_exp[0] is not None:
                    # scalar ring also runs the exps: pin after the final
                    # exp so the scheduler cannot hoist the collective-
                    # completion wait into the middle of the attention
                    add_dep_helper(d.ins, last_exp[0].ins,
                                   reason="zf load after attention exps")

            # ---- stream 1: phase-1 QKV projection units ----
            # group order [Q h0, K h0, V, Q h1, K h1]: the ("v", b, qc)
            # marker frees head-0 attention chunks 2 groups early, pulling
            # the first A2A trigger ahead of the attention tail
            def p1_qk(b, qc, h, xt_c, ql):
                for (w_sb, b_sb, dst) in ((wq_sb, bq_sb, qt_sb),
                                          (wk_sb, bk_sb, kt_sb)):
                    pp = ps.tile([128, QC], F32, tag="A",
                                 name=f"psp{b}_{qc}_{h}")
                    for mb in range(n_mb):
                        nc.tensor.matmul(pp, w_sb[:, h, mb, :],
                                         xt_c[:, mb, :],
                                         start=(mb == 0),
                                         stop=(mb == n_mb - 1))
                    nc.vector.tensor_scalar_add(
                        dst[:, b, h, ql], pp, b_sb[:, h:h + 1])
                    yield n_mb * QC

            def p1_gen():
                # b-alternating unit order spreads attention-chunk
                # availability (and so the attention phase's vector/scalar
                # work) evenly across the whole projection phase
                for qc in range(n_qc):
                    for b in range(batch):
                        ql = slice(qc * QC, (qc + 1) * QC)
                        xt_c = xin.tile([128, n_mb, QC], BF16, tag="xt",
                                        name=f"xt{b}_{qc}")
                        if b == 0 and qc == 0:
                            # quarter loads: leading matmuls start as soon as
                            # the first slices land on the cold queue
                            for mq in range(0, n_mb, 4):
                                nc.sync.dma_start(
                                    out=xt_c[:, mq:mq + 4],
                                    in_=xt_d[:, b, qc, mq:mq + 4])
                        else:
                            nc.sync.dma_start(out=xt_c, in_=xt_d[:, b, qc])
                        yield from p1_qk(b, qc, 0, xt_c, ql)
                        for pb4 in range(dpb):
                            pb = qc * dpb + pb4
                            pp = ps.tile([128, hpc * 128], F32, tag="A",
                                         name=f"psv{b}_{pb}")
                            for mb in range(n_mb):
                                nc.tensor.matmul(
                                    pp,
                                    xt_c[:, mb, pb4 * 128:(pb4 + 1) * 128],
                                    wv_sb[:, mb, :],
                                    start=(mb == 0), stop=(mb == n_mb - 1))
                            nc.vector.tensor_add(
                                v_sb[:, b, pb, :, :],
                                pp.rearrange("p (h d) -> p h d", h=hpc),
                                vb_sb.rearrange("p (h d) -> p h d", h=hpc))
                            yield n_mb * hpc * 128
                        markers.add(("v", b, qc))
                        for h in range(1, hpc):
                            yield from p1_qk(b, qc, h, xt_c, ql)
                        markers.add(("full", b, qc))

            # ---- stream 2: attention chunks, self-pipelined over k-tiles ----
            # qc-major order tracks unit availability; the h1 qc2 chunks are
            # deferred behind the h0 qc3 chunks so the first A2A trigger has
            # ~33us of attention PE work after it — enough to cover its
            # rendezvous skew + transfer AND the CC-engine serialization
            # before the second A2A
            chunks = [(h, b, qc) for qc in range(n_qc - 2)
                      for b in range(batch) for h in range(hpc)]
            chunks += [(0, 0, n_qc - 2), (0, 1, n_qc - 2),
                       (0, 0, n_qc - 1), (0, 1, n_qc - 1),
                       (1, 0, n_qc - 2), (1, 1, n_qc - 2),
                       (1, 0, n_qc - 1), (1, 1, n_qc - 1)]
            LAG = 2
            pend = []
            chain = {}

            def emit_score(ci, h, b, qc, kb, panel):
                dg = kb - qc * dpb
                off = 128 * dg if dg > 0 else 0
                np_ = QC - off
                st = ps.tile([128, QC], F32, tag="st", bufs=3,
                             name=f"st{ci}_{kb}")
                nc.tensor.matmul(st[:, :np_],
                                 kt_sb[:, b, h, kb * 128:(kb + 1) * 128],
                                 qt_sb[:, b, h, qc * QC + off:(qc + 1) * QC],
                                 start=True, stop=True)
                last_exp[0] = nc.scalar.activation(panel[:, kb, off:],
                                                   st[:, :np_], Exp)
                if dg >= 0:
                    band = slice(off, off + 128)
                    nc.vector.tensor_mul(panel[:, kb, band],
                                         panel[:, kb, band],
                                         mk_sb[:, dg, band])
                return np_

            def emit_av(ci, h, b, qc, kb, n_b, panel, zp, acc):
                dg = kb - qc * dpb
                off = 128 * dg if dg > 0 else 0
                nc.tensor.matmul(zp[:, off:], v_sb[:, b, kb, h, :],
                                 panel[:, kb, off:],
                                 start=(kb == 0), stop=(kb == n_b - 1),
                                 skip_group_check=True)
                # denominator partials: acc = sum_kb exp panel (masked rows
                # of diag tiles contribute only their valid [off:] columns)
                if kb == 0:
                    nc.vector.tensor_copy(acc, panel[:, 0, :])
                else:
                    nc.vector.tensor_add(acc[:, off:], acc[:, off:],
                                         panel[:, kb, off:])
                return QC - off

            def emit_ship(ci):
                # reduce+broadcast the denominators with one all-ones matmul,
                # then normalize the full z psum into the A2A staging tile
                h, b, qc, zp, acc = chain.pop(ci)
                accb = work.tile([128, QC], BF16, tag="accb", bufs=2,
                                 name=f"accb{ci}")
                nc.vector.tensor_copy(accb, acc)
                dps = ps.tile([128, QC], F32, tag="A", bufs=2, name=f"dps{ci}")
                nc.tensor.matmul(dps, ones_sb, accb, start=True, stop=True)
                rb = work.tile([128, QC], F32, tag="rb", bufs=2, name=f"rb{ci}")
                nc.vector.reciprocal_approx_fast(out=rb, in_=dps)
                zsb = work.tile([128, QC], BF16, tag="zsb", bufs=2,
                                name=f"zsb{ci}")
                nc.vector.tensor_mul(zsb, zp, rb)
                shard = b * n_qc + qc
                nc.sync.dma_start(out=a2a_in[h][shard], in_=zsb)
                ship_n[h] += 1
                if ship_n[h] == batch * n_qc:
                    emit_cc(h)
                return QC

            def attn_gen():
                for ci, (h, b, qc) in enumerate(chunks):
                    gate = ("v", b, qc) if h == 0 else ("full", b, qc)
                    while gate not in markers:
                        yield ("blocked", gate)
                    n_b = (qc + 1) * dpb
                    panel = work.tile([128, n_kb, QC], BF16, tag="pt", bufs=2,
                                      name=f"pt{ci}")
                    zp = ps.tile([128, QC], F32, tag="z", bufs=3,
                                 name=f"zp{ci}")
                    acc = work.tile([128, QC], F32, tag="acc", bufs=2,
                                    name=f"acc{ci}")
                    for k in range(n_b + LAG):
                        r = 0
                        if k < n_b:
                            r += emit_score(ci, h, b, qc, k, panel)
                        if 0 <= k - LAG < n_b:
                            r += emit_av(ci, h, b, qc, k - LAG, n_b, panel,
                                         zp, acc)
                        if k == 3 and pend:
                            r += emit_ship(pend.pop(0))
                        yield r
                    chain[ci] = (h, b, qc, zp, acc)
                    pend.append(ci)
                while pend:
                    yield emit_ship(pend.pop(0))
                # W_O chunks 0/1 into the long-freed xt slots on the sync
                # ring (all ships already issued ahead of them)
                for mc in range(min(2, n_mc)):
                    t = xin.tile([128, n_heads, MC], BF16, tag="xt",
                                 name=f"wo{mc}")
                    nc.sync.dma_start(
                        out=t, in_=wo_d[:, :, mc * MC:(mc + 1) * MC])
                    wo_tiles[mc] = t
                # first A2A's zf tiles on the gpsimd ring (reaches here right
                # after the tail denominator work)
                load_zf(0, nc.gpsimd)

            # ---- emission driver: fraction-paced interleave ----
            def drive(specs):
                gens = [g for g, _ in specs]
                totals = [float(t) for _, t in specs]
                spent = [0.0] * len(specs)
                alive = [True] * len(specs)
                blocked = [None] * len(specs)
                while any(alive):
                    cands = [i for i in range(len(specs)) if alive[i] and
                             (blocked[i] is None or blocked[i] in markers)]
                    assert cands, "emission driver deadlock"
                    i = min(cands, key=lambda j: spent[j] / totals[j])
                    blocked[i] = None
                    try:
                        item = next(gens[i])
                    except StopIteration:
                        alive[i] = False
                        continue
                    if isinstance(item, tuple):
                        blocked[i] = item[1]
                    else:
                        spent[i] += item

            p1_rows = batch * n_qc * (2 * hpc * n_mb * QC
                                      + dpb * n_mb * hpc * 128)
            at_rows = 0
            for (h, b, qc) in chunks:
                n_b = (qc + 1) * dpb
                for kb in range(n_b):
                    dg = kb - qc * dpb
                    off = 128 * dg if dg > 0 else 0
                    at_rows += 2 * (QC - off)
                at_rows += QC
            drive([(p1_gen(), p1_rows), (attn_gen(), at_rows)])

            # scalar-queue order at attention end: W_O chunks 2/3 into the
            # released qt/kt slots (no collective wait; pass E needs them
            # mid-pass), then zf odds (which wait on cc1). Pinned after the
            # exps so the scheduler cannot hoist the waits into attention.
            for mc, tg in zip(range(2, n_mc), ("qt_sb", "kt_sb")):
                t = stat.tile([128, n_heads, MC], BF16, tag=tg, name=f"wo{mc}")
                d = nc.scalar.dma_start(out=t,
                                        in_=wo_d[:, :, mc * MC:(mc + 1) * MC])
                if last_exp[0] is not None:
                    add_dep_helper(d.ins, last_exp[0].ins,
                                   reason="wo load after attention exps")
                wo_tiles[mc] = t
            for h in range(1, hpc):
                load_zf(h, nc.scalar)

            # ---- phase 3: output projection, two passes ----
            # Pass E runs the first-A2A heads for ALL output tiles staged to
            # SBUF in bf16; pass O adds the remaining heads once cc1's zf
            # tiles have arrived. (global head g = i*hpc + h for source
            # core i, local head h; zf is [128, h, i, rows])
            late_hi = [(h, i) for h in range(1, hpc) for i in range(n_cores)]
            osbe = stat.tile([128, n_mc * n_pb, MC], BF16)
            tiles3 = [(mc, pb) for mc in range(n_mc) for pb in range(n_pb)]
            for n, (mc, pb) in enumerate(tiles3):
                tg, bf = (("A", 2), ("st", 3))[n % 2]
                pp = ps.tile([128, MC], F32, tag=tg, bufs=bf,
                             name=f"pse{mc}_{pb}")
                for i in range(n_cores):
                    nc.tensor.matmul(
                        pp, zf_sb[:, 0, i, pb * 128:(pb + 1) * 128],
                        wo_tiles[mc][:, i * hpc, :],
                        start=(i == 0), stop=(i == n_cores - 1))
                nc.vector.tensor_copy(osbe[:, n, :], pp)
            for n, (mc, pb) in enumerate(tiles3):
                tg, bf = (("A", 2), ("st", 3))[n % 2]
                pp = ps.tile([128, MC], F32, tag=tg, bufs=bf,
                             name=f"pso{mc}_{pb}")
                for j, (h, i) in enumerate(late_hi):
                    nc.tensor.matmul(
                        pp, zf_sb[:, h, i, pb * 128:(pb + 1) * 128],
                        wo_tiles[mc][:, i * hpc + h, :],
                        start=(j == 0), stop=(j == len(late_hi) - 1))
                osb = work.tile([128, MC], BF16, tag="osb", bufs=2,
                                name=f"osb{mc}_{pb}")
                nc.vector.tensor_add(osb, pp, osbe[:, n, :])
                ml = slice(mc * MC, (mc + 1) * MC)
                # alternate output rings so the final flush isn't serialized
                oeng = nc.sync if n % 2 == 0 else nc.scalar
                oeng.dma_start(out=out_d[pb * 128:(pb + 1) * 128, ml],
                               in_=osb)

    nc.compile()
    return nc


def make_in_maps(inputs, cfg=FULL):
    c = _derived(cfg)
    hpc, QC = c["hpc"], c["qc_size"]
    n_mb, n_dg = c["n_mb"], c["n_dg"]
    d_model, seq, batch = c["d_model"], c["seq"], c["batch"]
    residual = np.asarray(inputs["residual"], np.float32)
    W_Q = np.asarray(inputs["W_Q"], np.float32)
    W_K = np.asarray(inputs["W_K"], np.float32)
    W_V = np.asarray(inputs["W_V"], np.float32)
    W_O = np.asarray(inputs["W_O"], np.float32)
    b_Q = np.asarray(inputs["b_Q"], np.float32)
    b_K = np.asarray(inputs["b_K"], np.float32)
    b_V = np.asarray(inputs["b_V"], np.float32)
    scale = 1.0 / ATTN_SCALE

    # X^T packed per q-chunk [128, batch, n_qc, n_mb, QC]:
    # [p, b, qc, mb, s'] = residual[b, qc*QC+s', mb*128+p]
    n_qc = c["n_qc"]
    xt = np.ascontiguousarray(
        residual.reshape(batch, n_qc, QC, n_mb, 128).transpose(4, 0, 1, 3, 2)
    ).astype(NP_BF16)
    # W_O packed [128, n_heads, d_model]
    wo = np.ascontiguousarray(
        W_O.transpose(1, 0, 2)).astype(NP_BF16)
    # causal {0,1} masks packed [128, n_dg, QC]
    masks = np.zeros((128, n_dg, QC), np.float32)
    pk = np.arange(128)[:, None]
    fq = np.arange(QC)[None, :]
    for dg in range(n_dg):
        masks[:, dg, :] = (fq >= pk + 128 * dg).astype(np.float32)
    masks = masks.astype(NP_BF16)

    in_maps = []
    for core in range(c["n_cores"]):
        hs = slice(core * hpc, (core + 1) * hpc)
        # [128, hpc, n_mb, 128]: [p, h, mb, d] = W[h, mb*128+p, d]
        wq = np.ascontiguousarray(
            (W_Q[hs] * scale).reshape(hpc, n_mb, 128, 128).transpose(2, 0, 1, 3)
        ).astype(NP_BF16)
        wk = np.ascontiguousarray(
            W_K[hs].reshape(hpc, n_mb, 128, 128).transpose(2, 0, 1, 3)
        ).astype(NP_BF16)
        # [128, n_mb, hpc*128]: [p, mb, (h d)] = W_V[h, mb*128+p, d]
        wv = np.ascontiguousarray(
            W_V[hs].reshape(hpc, n_mb, 128, 128).transpose(2, 1, 0, 3)
            .reshape(128, n_mb, hpc * 128)).astype(NP_BF16)
        bq = np.ascontiguousarray((b_Q[hs] * scale).T).astype(np.float32)
        bk = np.ascontiguousarray(b_K[hs].T).astype(np.float32)
        bv = np.ascontiguousarray(b_V[hs].reshape(hpc * 128)).astype(np.float32)
        in_maps.append({
            "xt": xt, "wq": wq, "wk": wk, "wv": wv, "wo": wo,
            "bq": bq, "bk": bk, "bv": bv, "mk": masks,
        })
    return in_maps


def assemble_output(inputs, shards, cfg=FULL):
    c = _derived(cfg)
    residual = np.asarray(inputs["residual"], np.float32)
    b_O = np.asarray(inputs["b_O"], np.float32)
    out = np.concatenate([np.asarray(s).astype(np.float32) for s in shards],
                         axis=0)
    out = out.reshape(c["batch"], c["seq"], c["d_model"]) + b_O
    return residual, out.astype(np.float32)


_NC_CACHE = {}


def _get_nc():
    if "nc" not in _NC_CACHE:
        _NC_CACHE["nc"] = build_graph(FULL)
    return _NC_CACHE["nc"]


def run(inputs, trace=False):
    nc = _get_nc()
    in_maps = make_in_maps(inputs, FULL)
    try:
        res = run_bass_kernel_spmd(nc, in_maps, list(range(FULL["n_cores"])),
                                   trace=trace)
    except Exception:
        # a previous bad run can leave the remote device wedged for one
        # attempt; give it a moment and retry once
        import time
        time.sleep(60)
        res = run_bass_kernel_spmd(nc, in_maps, list(range(FULL["n_cores"])),
                                   trace=trace)
    shards = [res.results[i]["out"] for i in range(FULL["n_cores"])]
    residual, out = assemble_output(inputs, shards, FULL)
    return (residual, out), res


def kernel(**inputs):
    (residual, out), _ = run(inputs, trace=False)
    return (residual, out)
